# revision 19
# baseline (speedup 1.0000x reference)
"""GAT attention kernel for 8 trn2 NeuronCores (Bass/Tile), bf16 pipeline.

Math (restructured from the reference):
    wa1 = W @ a1, wa2 = W @ a2                      (host, fp64)
    s2[n,k] = x0[n]·wa1 + x[n,k]·wa2 + mask[n,k]    (mask = 0 or -700)
    p       = exp(leaky_relu(s2, 0.2))              (masked entries underflow to 0,
                                                     exactly matching the -9e15 mask)
    att     = p / sum_k p
    xbar[n] = sum_k att[n,k] * x[n,k,:]
    out     = elu((xbar + x0) @ W)
    elu(z)  = min(exp(z)-1+relu(z), relu(z))

Sharding: node dim N padded 50000 -> 50176 = 8 cores * 49 tiles * 128 rows.
Per 128-row tile the 2048 (n,k) pairs form 16 blocks of [128 nk-rows, 128 feat]
held as x_tile[:, b*128:(b+1)*128] (host pre-permutes x; all data bf16).
Padded rows are shipped UNMASKED (mask=0) so their softmax is uniform over
zero vectors -> output 0, no inf/NaN.

Per tile:
  DVE : 17x tensor_tensor_reduce (si + 16 score blocks; the per-block
        reduction is SEEDED with si+mask via the scalar-init AP, fusing the
        bias add), lrelu STT, reciprocal (reads PSUM), att STT, attseg TT
  PE  : si scatter + mask (2 accumulating matmuls), Z group-sum (SEG),
        RZ broadcast (E8), x0^T identity + 16 xbar matmuls (PSUM acc),
        final (xbar+x0)^T.T @ W          -- all bf16, single-pass + FWL
  ACT : si_s copy, Exp(p), ST copy, Exp/Relu of final
  GPS : Dt = SEG8*si, elu tail (e-1+r, min)
"""

import numpy as np

N, K, F = 50000, 16, 128
ALPHA = 0.2
NCORES = 8
TILE = 128
NTILES = 49
RPC = TILE * NTILES          # rows per core = 6272
BPT = K                      # nk-blocks per tile = 16
XCOLS = BPT * F + F + K      # x blocks + x0 + mask = 2192
MASKVAL = -700.0             # exp(0.2*(s-700)) underflows to 0 for any real s
EPS = 1e-12

_NC_CACHE = {}


def _consts_np():
    import ml_dtypes
    p = np.arange(128)
    j8 = np.arange(8)
    b16 = np.arange(16)
    ident = np.eye(128, dtype=np.float32)
    # Cm[n, q] = 1 iff n%8 == q//16   (si scatter: si_s[q,b] = si[8b + q//16])
    Cm = (p[:, None] % 8 == p[None, :] // 16).astype(np.float32)
    # SEGBIG[q, 8b+j] = 1 iff j == q//16  (pattern repeats over b)
    segbig = (p[:, None] // 16 == (p[None, :] % 8)).astype(np.float32)
    # E8[j, q] = 1 iff q//16 == j (rows 8..127 zero; used as lhsT [8,128])
    e8 = ((p[:, None] < 8) & (p[None, :] // 16 == p[:, None])).astype(np.float32)
    # SEG[q, j] = 1 iff q//16 == j   [128, 8]
    seg = (p[:, None] // 16 == j8[None, :]).astype(np.float32)
    # SEG8[n, b] = 1 iff n//8 == b   [128, 16]
    seg8 = (p[:, None] // 8 == b16[None, :]).astype(np.float32)
    return np.concatenate([ident, Cm, segbig, e8, seg, seg8], axis=1)  # [128, 536]


def _consts_full_np(W, a):
    import ml_dtypes
    bf16 = ml_dtypes.bfloat16
    W64 = np.asarray(W, np.float64)
    wa1 = (W64 @ np.asarray(a[:F, 0], np.float64)).astype(np.float32)
    wa2 = (W64 @ np.asarray(a[F:, 0], np.float64)).astype(np.float32)
    wa1_rep = np.broadcast_to(wa1[None, :], (128, F))
    wa2_rep = np.broadcast_to(wa2[None, :], (128, F))
    return np.ascontiguousarray(
        np.concatenate(
            [_consts_np(), np.asarray(W, np.float32), wa1_rep, wa2_rep,
             wa1[:, None]], axis=1)
    ).astype(bf16)  # [128, 921]


def _build_nc(ntiles=NTILES, finalize=True):
    import concourse.mybir as mybir
    import concourse.tile as tile
    from concourse import bacc

    fp = mybir.dt.float32
    bf = mybir.dt.bfloat16
    AF = mybir.ActivationFunctionType
    OP = mybir.AluOpType

    nc = bacc.Bacc("TRN2")
    xd = nc.dram_tensor("xd", [ntiles, 128, XCOLS], bf, kind="ExternalInput")
    cst = nc.dram_tensor("cst", [128, 921], bf, kind="ExternalInput")
    yd = nc.dram_tensor("yd", [ntiles, 128, F], bf, kind="ExternalOutput")

    with tile.TileContext(nc) as tc:
        with (
            tc.tile_pool(name="const", bufs=1) as constp,
            tc.tile_pool(name="xin", bufs=7) as xin,
            tc.tile_pool(name="small", bufs=4) as small,
            tc.tile_pool(name="big", bufs=3) as big,
            tc.tile_pool(name="yout", bufs=3) as yout,
            # one PSUM pool; per-tag bufs: si 1 + Z 2 + RZrep 2 + mm 3 = 8 banks
            tc.tile_pool(name="ps", bufs=1, space="PSUM") as ps,
        ):
            consts = constp.tile([128, 921], bf)
            nc.sync.dma_start(out=consts, in_=cst[:, :])
            IDENT = consts[:, 0:128]
            Cm = consts[:, 128:256]
            SEGBIG = consts[:, 256:384]
            E8 = consts[:, 384:512]
            SEG = consts[:, 512:520]
            SEG8 = consts[:, 520:536]
            W_sb = consts[:, 536:664]
            wa1_rep = consts[:, 664:792]
            wa2_rep = consts[:, 792:920]
            wa1_col = consts[:, 920:921]

            # ------------- software-pipelined tile loop -------------
            #   load(t) | score(t-2) | mask(t-3) | recip(t-4) | out(t-5)
            st = {}

            def phase_load(t):
                xall = xin.tile([128, XCOLS], bf, tag="x")
                nc.sync.dma_start(out=xall, in_=xd[t])
                st[t] = {"xall": xall}

            def phase_score(t):
                d = st[t]
                xall = d["xall"]
                x0T_tile = xall[:, BPT * F:BPT * F + F]
                adjm = xall[:, BPT * F + F:XCOLS]
                # si[n] = x0[n]·wa1 on PE: lhsT = x0T [f, n], rhs = wa1 col
                si_ps = ps.tile([128, K + 1], fp, tag="si", bufs=1)
                nc.tensor.matmul(si_ps[:, K:K + 1], lhsT=x0T_tile, rhs=wa1_col,
                                 start=True, stop=True, skip_group_check=True)
                Dt = small.tile([128, K], bf, tag="D")
                nc.vector.tensor_scalar_mul(out=Dt, in0=SEG8,
                                            scalar1=si_ps[:, K:K + 1])
                nc.tensor.matmul(si_ps[:, 0:K], lhsT=Cm, rhs=Dt, start=True,
                                 stop=False, skip_group_check=True)
                nc.tensor.matmul(si_ps[:, 0:K], lhsT=IDENT, rhs=adjm,
                                 start=False, stop=True, skip_group_check=True)
                si_s = small.tile([128, K], fp, tag="si_s")
                nc.scalar.activation(out=si_s, in_=si_ps[:, 0:K], func=AF.Copy)

                prod = big.tile([128, BPT * F], bf, tag="prod")
                s = small.tile([128, K], fp, tag="s")
                for b in range(BPT):
                    nc.vector.scalar_tensor_tensor(
                        out=prod[:, b * F:(b + 1) * F],
                        in0=xall[:, b * F:(b + 1) * F], scalar=1.0,
                        in1=wa2_rep, op0=OP.mult, op1=OP.mult,
                        accum_out=s[:, b:b + 1],
                    )
                s2 = small.tile([128, K], fp, tag="s2")
                nc.gpsimd.tensor_add(out=s2, in0=s, in1=si_s)
                d["s2"] = s2

            def phase_mask(t):
                d = st[t]
                lp = small.tile([128, K], fp, tag="lp")
                nc.gpsimd.tensor_scalar(
                    out=lp, in0=d["s2"], scalar1=0.0, scalar2=1.0,
                    op0=OP.max, op1=OP.mult,
                )
                ln = small.tile([128, K], fp, tag="ln")
                nc.gpsimd.tensor_scalar(
                    out=ln, in0=d["s2"], scalar1=0.0, scalar2=ALPHA,
                    op0=OP.min, op1=OP.mult,
                )
                ls = small.tile([128, K], fp, tag="ls")
                nc.gpsimd.tensor_add(out=ls, in0=lp, in1=ln)
                p_s = small.tile([128, K], bf, tag="p_s")
                nc.scalar.activation(out=p_s, in_=ls, func=AF.Exp)
                Z_ps = ps.tile([8, K], fp, tag="Z", bufs=2)
                nc.tensor.matmul(Z_ps, lhsT=SEG, rhs=p_s, start=True, stop=True)
                d["p_s"] = p_s
                d["Z_ps"] = Z_ps

            def phase_recip(t):
                d = st[t]
                RZ = small.tile([8, K], bf, tag="RZ")
                with nc.allow_low_precision(reason="RZ bf16 feeds bf16 matmul"):
                    nc.vector.reciprocal(RZ, d["Z_ps"])
                RZrep_ps = ps.tile([128, K], fp, tag="RZrep", bufs=2)
                nc.tensor.matmul(RZrep_ps, lhsT=E8[0:8, :], rhs=RZ,
                                 start=True, stop=True)
                d["RZrep"] = RZrep_ps

            def phase_out(t):
                d = st[t]
                xall = d["xall"]
                x0T_tile = xall[:, BPT * F:BPT * F + F]
                att = small.tile([128, K], bf, tag="att")
                nc.vector.scalar_tensor_tensor(
                    out=att, in0=d["p_s"], scalar=EPS, in1=d["RZrep"],
                    op0=OP.add, op1=OP.mult,
                )
                attseg = big.tile([128, 128], bf, tag="attseg")
                att_bc = att.rearrange("p (b o) -> p b o", o=1).to_broadcast([128, K, 8])
                nc.vector.tensor_mul(
                    out=attseg.rearrange("p (b j) -> p b j", j=8),
                    in0=SEGBIG.rearrange("p (b j) -> p b j", j=8),
                    in1=att_bc,
                )
                xbarT_ps = ps.tile([128, 128], fp, tag="mm", bufs=3)
                nc.tensor.matmul(xbarT_ps, lhsT=IDENT, rhs=x0T_tile,
                                 start=True, stop=False, skip_group_check=True)
                for b in range(BPT):
                    nc.tensor.matmul(
                        xbarT_ps[:, 8 * b:8 * b + 8],
                        lhsT=xall[:, b * F:(b + 1) * F],
                        rhs=attseg[:, 8 * b:8 * b + 8],
                        start=False, stop=(b == BPT - 1), skip_group_check=True,
                    )
                ST_sb = big.tile([128, 128], bf, tag="ST")
                nc.scalar.activation(out=ST_sb, in_=xbarT_ps, func=AF.Copy)
                zfin_ps = ps.tile([128, 128], fp, tag="mm", bufs=3)
                nc.tensor.matmul(zfin_ps, lhsT=ST_sb, rhs=W_sb, start=True, stop=True)
                e_sb = big.tile([128, 128], bf, tag="e")
                nc.scalar.activation(out=e_sb, in_=zfin_ps, func=AF.Exp)
                r_sb = big.tile([128, 128], bf, tag="r")
                nc.scalar.activation(out=r_sb, in_=zfin_ps, func=AF.Relu)
                # y = elu = min(e - 1 + r, r)
                tt_sb = big.tile([128, 128], bf, tag="tt")
                nc.vector.scalar_tensor_tensor(
                    out=tt_sb, in0=e_sb, scalar=-1.0, in1=r_sb,
                    op0=OP.add, op1=OP.add,
                )
                y_sb = yout.tile([128, 128], bf, tag="y")
                nc.vector.tensor_tensor(out=y_sb, in0=tt_sb, in1=r_sb, op=OP.min)
                nc.sync.dma_start(out=yd[t], in_=y_sb)
                del st[t]

            for r in range(ntiles + 5):
                if r < ntiles:
                    phase_load(r)
                if 0 <= r - 2 < ntiles:
                    phase_score(r - 2)
                if 0 <= r - 3 < ntiles:
                    phase_mask(r - 3)
                if 0 <= r - 4 < ntiles:
                    phase_recip(r - 4)
                if 0 <= r - 5 < ntiles:
                    phase_out(r - 5)

    if finalize:
        nc.finalize()
    return nc


def _get_nc(ntiles=NTILES):
    if ntiles not in _NC_CACHE:
        _NC_CACHE[ntiles] = _build_nc(ntiles)
    return _NC_CACHE[ntiles]


def _shard_inputs(orignal_x, x, adj, W, a, ncores=NCORES, ntiles=NTILES):
    import ml_dtypes
    bf16 = ml_dtypes.bfloat16
    f32 = np.float32
    rpc = TILE * ntiles
    n_used = rpc * ncores
    x = np.asarray(x, f32)
    x0 = np.asarray(orignal_x, f32)
    adj = np.asarray(adj, np.int32)
    consts = _consts_full_np(np.asarray(W, f32), np.asarray(a, f32))
    n = x.shape[0]

    in_maps = []
    for c in range(ncores):
        lo = c * rpc
        hi = min((c + 1) * rpc, n)
        rows = hi - lo
        xc = x[lo:hi]
        x0c = x0[lo:hi]
        # mask: 0 where active, -700 where masked; padded rows UNMASKED (0)
        mc = np.where(adj[lo:hi] > 0, 0.0, MASKVAL).astype(f32)
        if rows < rpc:
            pad = rpc - rows
            xc = np.concatenate([xc, np.zeros((pad, K, F), f32)])
            x0c = np.concatenate([x0c, np.zeros((pad, F), f32)])
            mc = np.concatenate([mc, np.zeros((pad, K), f32)])
        xdev = np.empty((ntiles, 128, XCOLS), bf16)
        xdev[:, :, :BPT * F] = xc.reshape(ntiles, BPT, 128, F).transpose(
            0, 2, 1, 3).reshape(ntiles, 128, BPT * F).astype(bf16)
        xdev[:, :, BPT * F:BPT * F + F] = x0c.reshape(
            ntiles, 128, F).transpose(0, 2, 1).astype(bf16)
        xdev[:, :, BPT * F + F:] = mc.reshape(ntiles, BPT, 128).transpose(
            0, 2, 1).astype(bf16)
        in_maps.append({
            "xd": xdev,
            "cst": consts,
        })
    assert n <= n_used
    return in_maps


_LAST_RESULTS = None


def kernel(orignal_x, x, adj, W, a):
    import os
    os.environ.setdefault("JAX_PLATFORMS", "")
    from concourse.bass_utils import run_bass_kernel_spmd

    global _LAST_RESULTS
    nc = _get_nc()
    in_maps = _shard_inputs(orignal_x, x, adj, W, a)
    res = run_bass_kernel_spmd(nc, in_maps, list(range(NCORES)))
    _LAST_RESULTS = res
    y = np.concatenate([r["yd"].reshape(RPC, F) for r in res.results], axis=0)
    return np.ascontiguousarray(y[:N].astype(np.float32))


# revision 20
# speedup vs baseline: 1.5902x; 1.5902x over previous
"""GAT attention kernel for 8 trn2 NeuronCores (Bass/Tile), bf16 pipeline.

Math (restructured from the reference):
    wa1 = W @ a1, wa2 = W @ a2                      (host, fp64)
    s2[n,k] = x0[n]·wa1 + x[n,k]·wa2 + mask[n,k]    (mask = 0 or -700)
    p       = exp(leaky_relu(s2, 0.2))              (masked entries underflow to 0,
                                                     exactly matching the -9e15 mask)
    att     = p / sum_k p
    xbar[n] = sum_k att[n,k] * x[n,k,:]
    out     = elu((xbar + x0) @ W)
    elu(z)  = min(exp(z)-1+relu(z), relu(z))

Sharding: node dim N padded 50000 -> 50176 = 8 cores * 49 tiles * 128 rows.
Per 128-row tile the 2048 (n,k) pairs form 16 blocks of [128 nk-rows, 128 feat]
held as x_tile[:, b*128:(b+1)*128] (host pre-permutes x; all data bf16).
Padded rows are shipped UNMASKED (mask=0) so their softmax is uniform over
zero vectors -> output 0, no inf/NaN.

Per tile:
  DVE : 17x tensor_tensor_reduce (si + 16 score blocks; the per-block
        reduction is SEEDED with si+mask via the scalar-init AP, fusing the
        bias add), lrelu STT, reciprocal (reads PSUM), att STT, attseg TT
  PE  : si scatter + mask (2 accumulating matmuls), Z group-sum (SEG),
        RZ broadcast (E8), x0^T identity + 16 xbar matmuls (PSUM acc),
        final (xbar+x0)^T.T @ W          -- all bf16, single-pass + FWL
  ACT : si_s copy, Exp(p), ST copy, Exp/Relu of final
  GPS : Dt = SEG8*si, elu tail (e-1+r, min)
"""

import numpy as np

N, K, F = 50000, 16, 128
ALPHA = 0.2
NCORES = 8
TILE = 128
NTILES = 49
RPC = TILE * NTILES          # rows per core = 6272
BPT = K                      # nk-blocks per tile = 16
XCOLS = BPT * F + F + K      # x blocks + x0 + mask = 2192
MASKVAL = -700.0             # exp(0.2*(s-700)) underflows to 0 for any real s
EPS = 1e-12

_NC_CACHE = {}


def _consts_np():
    import ml_dtypes
    p = np.arange(128)
    j8 = np.arange(8)
    b16 = np.arange(16)
    ident = np.eye(128, dtype=np.float32)
    # Cm[n, q] = 1 iff n%8 == q//16   (si scatter: si_s[q,b] = si[8b + q//16])
    Cm = (p[:, None] % 8 == p[None, :] // 16).astype(np.float32)
    # SEGBIG[q, 8b+j] = 1 iff j == q//16  (pattern repeats over b)
    segbig = (p[:, None] // 16 == (p[None, :] % 8)).astype(np.float32)
    # E8[j, q] = 1 iff q//16 == j (rows 8..127 zero; used as lhsT [8,128])
    e8 = ((p[:, None] < 8) & (p[None, :] // 16 == p[:, None])).astype(np.float32)
    # SEG[q, j] = 1 iff q//16 == j   [128, 8]
    seg = (p[:, None] // 16 == j8[None, :]).astype(np.float32)
    # SEG8[n, b] = 1 iff n//8 == b   [128, 16]
    seg8 = (p[:, None] // 8 == b16[None, :]).astype(np.float32)
    return np.concatenate([ident, Cm, segbig, e8, seg, seg8], axis=1)  # [128, 536]


def _consts_full_np(W, a):
    import ml_dtypes
    bf16 = ml_dtypes.bfloat16
    W64 = np.asarray(W, np.float64)
    wa1 = (W64 @ np.asarray(a[:F, 0], np.float64)).astype(np.float32)
    wa2 = (W64 @ np.asarray(a[F:, 0], np.float64)).astype(np.float32)
    wa1_rep = np.broadcast_to(wa1[None, :], (128, F))
    wa2_rep = np.broadcast_to(wa2[None, :], (128, F))
    return np.ascontiguousarray(
        np.concatenate(
            [_consts_np(), np.asarray(W, np.float32), wa1_rep, wa2_rep,
             wa1[:, None]], axis=1)
    ).astype(bf16)  # [128, 921]


def _build_nc(ntiles=NTILES, finalize=True):
    import concourse.mybir as mybir
    import concourse.tile as tile
    from concourse import bacc

    fp = mybir.dt.float32
    bf = mybir.dt.bfloat16
    AF = mybir.ActivationFunctionType
    OP = mybir.AluOpType

    nc = bacc.Bacc("TRN2")
    xd = nc.dram_tensor("xd", [ntiles, 128, XCOLS], bf, kind="ExternalInput")
    cst = nc.dram_tensor("cst", [128, 921], bf, kind="ExternalInput")
    yd = nc.dram_tensor("yd", [ntiles, 128, F], bf, kind="ExternalOutput")

    with tile.TileContext(nc) as tc:
        with (
            tc.tile_pool(name="const", bufs=1) as constp,
            tc.tile_pool(name="xin", bufs=7) as xin,
            tc.tile_pool(name="small", bufs=4) as small,
            tc.tile_pool(name="big", bufs=3) as big,
            tc.tile_pool(name="yout", bufs=3) as yout,
            # one PSUM pool; per-tag bufs: si 1 + Z 2 + RZrep 2 + mm 3 = 8 banks
            tc.tile_pool(name="ps", bufs=1, space="PSUM") as ps,
        ):
            consts = constp.tile([128, 921], bf)
            nc.sync.dma_start(out=consts, in_=cst[:, :])
            IDENT = consts[:, 0:128]
            Cm = consts[:, 128:256]
            SEGBIG = consts[:, 256:384]
            E8 = consts[:, 384:512]
            SEG = consts[:, 512:520]
            SEG8 = consts[:, 520:536]
            W_sb = consts[:, 536:664]
            wa1_rep = consts[:, 664:792]
            wa2_rep = consts[:, 792:920]
            wa1_col = consts[:, 920:921]

            # ------------- software-pipelined tile loop -------------
            #   load(t) | score(t-2) | mask(t-3) | recip(t-4) | out(t-5)
            st = {}

            def phase_load(t):
                xall = xin.tile([128, XCOLS], bf, tag="x")
                nc.sync.dma_start(out=xall, in_=xd[t])
                st[t] = {"xall": xall}

            def phase_score(t):
                d = st[t]
                xall = d["xall"]
                x0T_tile = xall[:, BPT * F:BPT * F + F]
                adjm = xall[:, BPT * F + F:XCOLS]
                # si[n] = x0[n]·wa1 on PE: lhsT = x0T [f, n], rhs = wa1 col
                si_ps = ps.tile([128, K + 1], fp, tag="si", bufs=2)
                nc.tensor.matmul(si_ps[:, K:K + 1], lhsT=x0T_tile, rhs=wa1_col,
                                 start=True, stop=True, skip_group_check=True)
                Dt = small.tile([128, K], bf, tag="D")
                nc.vector.tensor_scalar_mul(out=Dt, in0=SEG8,
                                            scalar1=si_ps[:, K:K + 1])
                nc.tensor.matmul(si_ps[:, 0:K], lhsT=Cm, rhs=Dt, start=True,
                                 stop=False, skip_group_check=True)
                nc.tensor.matmul(si_ps[:, 0:K], lhsT=IDENT, rhs=adjm,
                                 start=False, stop=True, skip_group_check=True)
                si_s = small.tile([128, K], fp, tag="si_s")
                nc.scalar.activation(out=si_s, in_=si_ps[:, 0:K], func=AF.Copy)

                prod = big.tile([128, BPT * F], bf, tag="prod")
                s = small.tile([128, K], fp, tag="s")
                for b in range(BPT):
                    nc.vector.scalar_tensor_tensor(
                        out=prod[:, b * F:(b + 1) * F],
                        in0=xall[:, b * F:(b + 1) * F], scalar=1.0,
                        in1=wa2_rep, op0=OP.mult, op1=OP.mult,
                        accum_out=s[:, b:b + 1],
                    )
                s2 = small.tile([128, K], fp, tag="s2")
                nc.gpsimd.tensor_add(out=s2, in0=s, in1=si_s)
                d["s2"] = s2

            def phase_mask(t):
                d = st[t]
                ls = small.tile([128, K], fp, tag="ls")
                nc.vector.scalar_tensor_tensor(
                    out=ls, in0=d["s2"], scalar=ALPHA, in1=d["s2"],
                    op0=OP.mult, op1=OP.max,
                )
                p_s = small.tile([128, K], bf, tag="p_s")
                nc.scalar.activation(out=p_s, in_=ls, func=AF.Exp)
                Z_ps = ps.tile([8, K], fp, tag="Z", bufs=2)
                nc.tensor.matmul(Z_ps, lhsT=SEG, rhs=p_s, start=True, stop=True)
                d["p_s"] = p_s
                d["Z_ps"] = Z_ps

            def phase_recip(t):
                d = st[t]
                RZ = small.tile([8, K], bf, tag="RZ")
                with nc.allow_low_precision(reason="RZ bf16 feeds bf16 matmul"):
                    nc.vector.reciprocal(RZ, d["Z_ps"])
                RZrep_ps = ps.tile([128, K], fp, tag="RZrep", bufs=2)
                nc.tensor.matmul(RZrep_ps, lhsT=E8[0:8, :], rhs=RZ,
                                 start=True, stop=True)
                d["RZrep"] = RZrep_ps

            def phase_out(t):
                d = st[t]
                xall = d["xall"]
                x0T_tile = xall[:, BPT * F:BPT * F + F]
                att = small.tile([128, K], bf, tag="att")
                nc.vector.scalar_tensor_tensor(
                    out=att, in0=d["p_s"], scalar=EPS, in1=d["RZrep"],
                    op0=OP.add, op1=OP.mult,
                )
                attseg = big.tile([128, 128], bf, tag="attseg")
                att_bc = att.rearrange("p (b o) -> p b o", o=1).to_broadcast([128, K, 8])
                nc.vector.tensor_mul(
                    out=attseg.rearrange("p (b j) -> p b j", j=8),
                    in0=SEGBIG.rearrange("p (b j) -> p b j", j=8),
                    in1=att_bc,
                )
                xbarT_ps = ps.tile([128, 128], fp, tag="mm", bufs=2)
                nc.tensor.matmul(xbarT_ps, lhsT=IDENT, rhs=x0T_tile,
                                 start=True, stop=False, skip_group_check=True)
                for b in range(BPT):
                    nc.tensor.matmul(
                        xbarT_ps[:, 8 * b:8 * b + 8],
                        lhsT=xall[:, b * F:(b + 1) * F],
                        rhs=attseg[:, 8 * b:8 * b + 8],
                        start=False, stop=(b == BPT - 1), skip_group_check=True,
                    )
                ST_sb = big.tile([128, 128], bf, tag="ST")
                nc.scalar.activation(out=ST_sb, in_=xbarT_ps, func=AF.Copy)
                zfin_ps = ps.tile([128, 128], fp, tag="mm", bufs=2)
                nc.tensor.matmul(zfin_ps, lhsT=ST_sb, rhs=W_sb, start=True, stop=True)
                e_sb = big.tile([128, 128], bf, tag="e")
                nc.scalar.activation(out=e_sb, in_=zfin_ps, func=AF.Exp)
                r_sb = big.tile([128, 128], bf, tag="r")
                nc.scalar.activation(out=r_sb, in_=zfin_ps, func=AF.Relu)
                # y = elu = min(e - 1 + r, r)
                tt_sb = big.tile([128, 128], bf, tag="tt")
                nc.vector.scalar_tensor_tensor(
                    out=tt_sb, in0=e_sb, scalar=-1.0, in1=r_sb,
                    op0=OP.add, op1=OP.add,
                )
                y_sb = yout.tile([128, 128], bf, tag="y")
                nc.vector.tensor_tensor(out=y_sb, in0=tt_sb, in1=r_sb, op=OP.min)
                nc.sync.dma_start(out=yd[t], in_=y_sb)
                del st[t]

            for r in range(ntiles + 5):
                if r < ntiles:
                    phase_load(r)
                if 0 <= r - 2 < ntiles:
                    phase_score(r - 2)
                if 0 <= r - 3 < ntiles:
                    phase_mask(r - 3)
                if 0 <= r - 4 < ntiles:
                    phase_recip(r - 4)
                if 0 <= r - 5 < ntiles:
                    phase_out(r - 5)

    if finalize:
        nc.finalize()
    return nc


def _get_nc(ntiles=NTILES):
    if ntiles not in _NC_CACHE:
        _NC_CACHE[ntiles] = _build_nc(ntiles)
    return _NC_CACHE[ntiles]


def _shard_inputs(orignal_x, x, adj, W, a, ncores=NCORES, ntiles=NTILES):
    import ml_dtypes
    bf16 = ml_dtypes.bfloat16
    f32 = np.float32
    rpc = TILE * ntiles
    n_used = rpc * ncores
    x = np.asarray(x, f32)
    x0 = np.asarray(orignal_x, f32)
    adj = np.asarray(adj, np.int32)
    consts = _consts_full_np(np.asarray(W, f32), np.asarray(a, f32))
    n = x.shape[0]

    in_maps = []
    for c in range(ncores):
        lo = c * rpc
        hi = min((c + 1) * rpc, n)
        rows = hi - lo
        xc = x[lo:hi]
        x0c = x0[lo:hi]
        # mask: 0 where active, -700 where masked; padded rows UNMASKED (0)
        mc = np.where(adj[lo:hi] > 0, 0.0, MASKVAL).astype(f32)
        if rows < rpc:
            pad = rpc - rows
            xc = np.concatenate([xc, np.zeros((pad, K, F), f32)])
            x0c = np.concatenate([x0c, np.zeros((pad, F), f32)])
            mc = np.concatenate([mc, np.zeros((pad, K), f32)])
        xdev = np.empty((ntiles, 128, XCOLS), bf16)
        xdev[:, :, :BPT * F] = xc.reshape(ntiles, BPT, 128, F).transpose(
            0, 2, 1, 3).reshape(ntiles, 128, BPT * F).astype(bf16)
        xdev[:, :, BPT * F:BPT * F + F] = x0c.reshape(
            ntiles, 128, F).transpose(0, 2, 1).astype(bf16)
        xdev[:, :, BPT * F + F:] = mc.reshape(ntiles, BPT, 128).transpose(
            0, 2, 1).astype(bf16)
        in_maps.append({
            "xd": xdev,
            "cst": consts,
        })
    assert n <= n_used
    return in_maps


_LAST_RESULTS = None


def kernel(orignal_x, x, adj, W, a):
    import os
    os.environ.setdefault("JAX_PLATFORMS", "")
    from concourse.bass_utils import run_bass_kernel_spmd

    global _LAST_RESULTS
    nc = _get_nc()
    in_maps = _shard_inputs(orignal_x, x, adj, W, a)
    res = run_bass_kernel_spmd(nc, in_maps, list(range(NCORES)))
    _LAST_RESULTS = res
    y = np.concatenate([r["yd"].reshape(RPC, F) for r in res.results], axis=0)
    return np.ascontiguousarray(y[:N].astype(np.float32))


# revision 22
# speedup vs baseline: 1.8213x; 1.1454x over previous
"""GAT attention kernel for 8 trn2 NeuronCores (Bass/Tile), bf16 pipeline.

Math (restructured from the reference):
    wa1 = W @ a1, wa2 = W @ a2                      (host, fp64)
    s2[n,k] = x0[n]·wa1 + x[n,k]·wa2 + mask[n,k]    (mask = 0 or -700)
    p       = exp(leaky_relu(s2, 0.2))              (masked entries underflow to 0,
                                                     exactly matching the -9e15 mask)
    att     = p / sum_k p
    xbar[n] = sum_k att[n,k] * x[n,k,:]
    out     = elu((xbar + x0) @ W)
    elu(z)  = min(exp(z)-1+relu(z), relu(z))

Sharding: node dim N padded 50000 -> 50176 = 8 cores * 49 tiles * 128 rows.
Per 128-row tile the 2048 (n,k) pairs form 16 blocks of [128 nk-rows, 128 feat]
held as x_tile[:, b*128:(b+1)*128] (host pre-permutes x; all data bf16).
Padded rows are shipped UNMASKED (mask=0) so their softmax is uniform over
zero vectors -> output 0, no inf/NaN.

Per tile:
  DVE : 17x tensor_tensor_reduce (si + 16 score blocks; the per-block
        reduction is SEEDED with si+mask via the scalar-init AP, fusing the
        bias add), lrelu STT, reciprocal (reads PSUM), att STT, attseg TT
  PE  : si scatter + mask (2 accumulating matmuls), Z group-sum (SEG),
        RZ broadcast (E8), x0^T identity + 16 xbar matmuls (PSUM acc),
        final (xbar+x0)^T.T @ W          -- all bf16, single-pass + FWL
  ACT : si_s copy, Exp(p), ST copy, Exp/Relu of final
  GPS : Dt = SEG8*si, elu tail (e-1+r, min)
"""

import numpy as np

N, K, F = 50000, 16, 128
ALPHA = 0.2
NCORES = 8
TILE = 128
NTILES = 49
RPC = TILE * NTILES          # rows per core = 6272
BPT = K                      # nk-blocks per tile = 16
XCOLS = BPT * F + F + K      # x blocks + x0 + mask = 2192
MASKVAL = -700.0             # exp(0.2*(s-700)) underflows to 0 for any real s
EPS = 1e-12

_NC_CACHE = {}


def _consts_np():
    import ml_dtypes
    p = np.arange(128)
    j8 = np.arange(8)
    b16 = np.arange(16)
    ident = np.eye(128, dtype=np.float32)
    # Cm[n, q] = 1 iff n%8 == q//16   (si scatter: si_s[q,b] = si[8b + q//16])
    Cm = (p[:, None] % 8 == p[None, :] // 16).astype(np.float32)
    # SEGBIG[q, 8b+j] = 1 iff j == q//16  (pattern repeats over b)
    segbig = (p[:, None] // 16 == (p[None, :] % 8)).astype(np.float32)
    # E8[j, q] = 1 iff q//16 == j (rows 8..127 zero; used as lhsT [8,128])
    e8 = ((p[:, None] < 8) & (p[None, :] // 16 == p[:, None])).astype(np.float32)
    # SEG[q, j] = 1 iff q//16 == j   [128, 8]
    seg = (p[:, None] // 16 == j8[None, :]).astype(np.float32)
    # SEG8[n, b] = 1 iff n//8 == b   [128, 16]
    seg8 = (p[:, None] // 8 == b16[None, :]).astype(np.float32)
    return np.concatenate([ident, Cm, segbig, e8, seg, seg8], axis=1)  # [128, 536]


def _consts_full_np(W, a):
    import ml_dtypes
    bf16 = ml_dtypes.bfloat16
    W64 = np.asarray(W, np.float64)
    wa1 = (W64 @ np.asarray(a[:F, 0], np.float64)).astype(np.float32)
    wa2 = (W64 @ np.asarray(a[F:, 0], np.float64)).astype(np.float32)
    wa1_rep = np.broadcast_to(wa1[None, :], (128, F))
    wa2_rep = np.broadcast_to(wa2[None, :], (128, F))
    return np.ascontiguousarray(
        np.concatenate(
            [_consts_np(), np.asarray(W, np.float32), wa1_rep, wa2_rep,
             wa1[:, None]], axis=1)
    ).astype(bf16)  # [128, 921]


def _build_nc(ntiles=NTILES, finalize=True):
    import concourse.mybir as mybir
    import concourse.tile as tile
    from concourse import bacc

    fp = mybir.dt.float32
    bf = mybir.dt.bfloat16
    AF = mybir.ActivationFunctionType
    OP = mybir.AluOpType

    nc = bacc.Bacc("TRN2")
    xd = nc.dram_tensor("xd", [ntiles, 128, XCOLS], bf, kind="ExternalInput")
    cst = nc.dram_tensor("cst", [128, 921], bf, kind="ExternalInput")
    yd = nc.dram_tensor("yd", [ntiles, 128, F], bf, kind="ExternalOutput")

    with tile.TileContext(nc) as tc:
        with (
            tc.tile_pool(name="const", bufs=1) as constp,
            tc.tile_pool(name="xin", bufs=7) as xin,
            tc.tile_pool(name="small", bufs=4) as small,
            tc.tile_pool(name="big", bufs=3) as big,
            tc.tile_pool(name="yout", bufs=3) as yout,
            # one PSUM pool; per-tag bufs: si 1 + Z 2 + RZrep 2 + mm 3 = 8 banks
            tc.tile_pool(name="ps", bufs=1, space="PSUM") as ps,
        ):
            consts = constp.tile([128, 921], bf)
            nc.sync.dma_start(out=consts, in_=cst[:, :])
            IDENT = consts[:, 0:128]
            Cm = consts[:, 128:256]
            SEGBIG = consts[:, 256:384]
            E8 = consts[:, 384:512]
            SEG = consts[:, 512:520]
            SEG8 = consts[:, 520:536]
            W_sb = consts[:, 536:664]
            wa1_rep = consts[:, 664:792]
            wa2_rep = consts[:, 792:920]
            wa1_col = consts[:, 920:921]

            # ------------- software-pipelined tile loop -------------
            #   load(t) | score(t-2) | mask(t-3) | recip(t-4) | out(t-5)
            st = {}

            def phase_load(t):
                xall = xin.tile([128, XCOLS], bf, tag="x")
                nc.sync.dma_start(out=xall, in_=xd[t])
                st[t] = {"xall": xall}

            def phase_score(t):
                d = st[t]
                xall = d["xall"]
                x0T_tile = xall[:, BPT * F:BPT * F + F]
                adjm = xall[:, BPT * F + F:XCOLS]
                # si[n] = x0[n]·wa1 on PE: lhsT = x0T [f, n], rhs = wa1 col
                si_ps = ps.tile([128, K + 1], fp, tag="si", bufs=2)
                nc.tensor.matmul(si_ps[:, K:K + 1], lhsT=x0T_tile, rhs=wa1_col,
                                 start=True, stop=True, skip_group_check=True)
                Dt = small.tile([128, K], bf, tag="D")
                nc.vector.tensor_scalar_mul(out=Dt, in0=SEG8,
                                            scalar1=si_ps[:, K:K + 1])
                nc.tensor.matmul(si_ps[:, 0:K], lhsT=Cm, rhs=Dt, start=True,
                                 stop=False, skip_group_check=True)
                nc.tensor.matmul(si_ps[:, 0:K], lhsT=IDENT, rhs=adjm,
                                 start=False, stop=True, skip_group_check=True)
                si_s = small.tile([128, K], fp, tag="si_s")
                nc.scalar.activation(out=si_s, in_=si_ps[:, 0:K], func=AF.Copy)

                prod = big.tile([128, BPT * F], bf, tag="prod")
                s = small.tile([128, K], fp, tag="s")
                for b in range(BPT):
                    nc.vector.scalar_tensor_tensor(
                        out=prod[:, b * F:(b + 1) * F],
                        in0=xall[:, b * F:(b + 1) * F], scalar=1.0,
                        in1=wa2_rep, op0=OP.mult, op1=OP.mult,
                        accum_out=s[:, b:b + 1],
                    )
                s2 = small.tile([128, K], fp, tag="s2")
                nc.gpsimd.tensor_add(out=s2, in0=s, in1=si_s)
                d["s2"] = s2

            def phase_mask(t):
                d = st[t]
                ls = small.tile([128, K], fp, tag="ls")
                nc.vector.scalar_tensor_tensor(
                    out=ls, in0=d["s2"], scalar=ALPHA, in1=d["s2"],
                    op0=OP.mult, op1=OP.max,
                )
                p_s = small.tile([128, K], bf, tag="p_s")
                nc.scalar.activation(out=p_s, in_=ls, func=AF.Exp)
                Z_ps = ps.tile([8, K], fp, tag="Z", bufs=2)
                nc.tensor.matmul(Z_ps, lhsT=SEG, rhs=p_s, start=True, stop=True)
                d["p_s"] = p_s
                d["Z_ps"] = Z_ps

            def phase_recip(t):
                d = st[t]
                RZ = small.tile([8, K], bf, tag="RZ")
                with nc.allow_low_precision(reason="RZ bf16 feeds bf16 matmul"):
                    nc.vector.reciprocal(RZ, d["Z_ps"])
                RZrep_ps = ps.tile([128, K], fp, tag="RZrep", bufs=2)
                nc.tensor.matmul(RZrep_ps, lhsT=E8[0:8, :], rhs=RZ,
                                 start=True, stop=True)
                d["RZrep"] = RZrep_ps

            def phase_out(t):
                d = st[t]
                xall = d["xall"]
                x0T_tile = xall[:, BPT * F:BPT * F + F]
                att = small.tile([128, K], bf, tag="att")
                nc.vector.scalar_tensor_tensor(
                    out=att, in0=d["p_s"], scalar=EPS, in1=d["RZrep"],
                    op0=OP.add, op1=OP.mult,
                )
                # previous tile's elu tail sits between att and attseg so the
                # DVE pipe-drain of att is hidden
                prev = st.get(t - 1)
                if prev is not None and "e2" in prev:
                    y_sb = yout.tile([128, 128], bf, tag="y")
                    nc.vector.scalar_tensor_tensor(
                        out=y_sb, in0=prev["e2"], scalar=-1.0, in1=prev["r"],
                        op0=OP.add, op1=OP.add,
                    )
                    nc.sync.dma_start(out=yd[t - 1], in_=y_sb)
                    del prev["e2"], prev["r"]
                attseg = big.tile([128, 128], bf, tag="attseg")
                att_bc = att.rearrange("p (b o) -> p b o", o=1).to_broadcast([128, K, 8])
                nc.vector.tensor_mul(
                    out=attseg.rearrange("p (b j) -> p b j", j=8),
                    in0=SEGBIG.rearrange("p (b j) -> p b j", j=8),
                    in1=att_bc,
                )
                xbarT_ps = ps.tile([128, 128], fp, tag="mm", bufs=2)
                nc.tensor.matmul(xbarT_ps, lhsT=IDENT, rhs=x0T_tile,
                                 start=True, stop=False, skip_group_check=True)
                for b in range(BPT):
                    nc.tensor.matmul(
                        xbarT_ps[:, 8 * b:8 * b + 8],
                        lhsT=xall[:, b * F:(b + 1) * F],
                        rhs=attseg[:, 8 * b:8 * b + 8],
                        start=False, stop=(b == BPT - 1), skip_group_check=True,
                    )
                ST_sb = big.tile([128, 128], bf, tag="ST")
                nc.scalar.activation(out=ST_sb, in_=xbarT_ps, func=AF.Copy)
                zfin_ps = ps.tile([128, 128], fp, tag="mm", bufs=2)
                nc.tensor.matmul(zfin_ps, lhsT=ST_sb, rhs=W_sb, start=True, stop=True)
                r_sb = big.tile([128, 128], bf, tag="r")
                nc.scalar.activation(out=r_sb, in_=zfin_ps, func=AF.Relu)
                rm_sb = big.tile([128, 128], fp, tag="rm")
                nc.scalar.activation(out=rm_sb, in_=zfin_ps, func=AF.Relu,
                                     scale=-1.0)
                e2_sb = big.tile([128, 128], bf, tag="e2")
                nc.scalar.activation(out=e2_sb, in_=rm_sb, func=AF.Exp,
                                     scale=-1.0)
                d["e2"] = e2_sb
                d["r"] = r_sb
                # y of this tile is emitted by phase_out(t+1) (or the drain
                # below for the final tile)

            for r in range(ntiles + 5):
                if r < ntiles:
                    phase_load(r)
                if 0 <= r - 2 < ntiles:
                    phase_score(r - 2)
                if 0 <= r - 3 < ntiles:
                    phase_mask(r - 3)
                if 0 <= r - 4 < ntiles:
                    phase_recip(r - 4)
                if 0 <= r - 5 < ntiles:
                    phase_out(r - 5)
            last = st[ntiles - 1]
            y_sb = yout.tile([128, 128], bf, tag="y")
            nc.vector.scalar_tensor_tensor(
                out=y_sb, in0=last["e2"], scalar=-1.0, in1=last["r"],
                op0=OP.add, op1=OP.add,
            )
            nc.sync.dma_start(out=yd[ntiles - 1], in_=y_sb)

    if finalize:
        nc.finalize()
    return nc


def _get_nc(ntiles=NTILES):
    if ntiles not in _NC_CACHE:
        _NC_CACHE[ntiles] = _build_nc(ntiles)
    return _NC_CACHE[ntiles]


def _shard_inputs(orignal_x, x, adj, W, a, ncores=NCORES, ntiles=NTILES):
    import ml_dtypes
    bf16 = ml_dtypes.bfloat16
    f32 = np.float32
    rpc = TILE * ntiles
    n_used = rpc * ncores
    x = np.asarray(x, f32)
    x0 = np.asarray(orignal_x, f32)
    adj = np.asarray(adj, np.int32)
    consts = _consts_full_np(np.asarray(W, f32), np.asarray(a, f32))
    n = x.shape[0]

    in_maps = []
    for c in range(ncores):
        lo = c * rpc
        hi = min((c + 1) * rpc, n)
        rows = hi - lo
        xc = x[lo:hi]
        x0c = x0[lo:hi]
        # mask: 0 where active, -700 where masked; padded rows UNMASKED (0)
        mc = np.where(adj[lo:hi] > 0, 0.0, MASKVAL).astype(f32)
        if rows < rpc:
            pad = rpc - rows
            xc = np.concatenate([xc, np.zeros((pad, K, F), f32)])
            x0c = np.concatenate([x0c, np.zeros((pad, F), f32)])
            mc = np.concatenate([mc, np.zeros((pad, K), f32)])
        xdev = np.empty((ntiles, 128, XCOLS), bf16)
        xdev[:, :, :BPT * F] = xc.reshape(ntiles, BPT, 128, F).transpose(
            0, 2, 1, 3).reshape(ntiles, 128, BPT * F).astype(bf16)
        xdev[:, :, BPT * F:BPT * F + F] = x0c.reshape(
            ntiles, 128, F).transpose(0, 2, 1).astype(bf16)
        xdev[:, :, BPT * F + F:] = mc.reshape(ntiles, BPT, 128).transpose(
            0, 2, 1).astype(bf16)
        in_maps.append({
            "xd": xdev,
            "cst": consts,
        })
    assert n <= n_used
    return in_maps


_LAST_RESULTS = None


def kernel(orignal_x, x, adj, W, a):
    import os
    os.environ.setdefault("JAX_PLATFORMS", "")
    from concourse.bass_utils import run_bass_kernel_spmd

    global _LAST_RESULTS
    nc = _get_nc()
    in_maps = _shard_inputs(orignal_x, x, adj, W, a)
    res = run_bass_kernel_spmd(nc, in_maps, list(range(NCORES)))
    _LAST_RESULTS = res
    y = np.concatenate([r["yd"].reshape(RPC, F) for r in res.results], axis=0)
    return np.ascontiguousarray(y[:N].astype(np.float32))


# revision 24
# speedup vs baseline: 2.2567x; 1.2390x over previous
"""GAT attention kernel for 8 trn2 NeuronCores (Bass/Tile), bf16 pipeline.

Math (restructured from the reference):
    wa1 = W @ a1, wa2 = W @ a2                      (host, fp64)
    s2[n,k] = x0[n]·wa1 + x[n,k]·wa2 + mask[n,k]    (mask = 0 or -700)
    p       = exp(leaky_relu(s2, 0.2))              (masked entries underflow to 0,
                                                     exactly matching the -9e15 mask)
    att     = p / sum_k p
    xbar[n] = sum_k att[n,k] * x[n,k,:]
    out     = elu((xbar + x0) @ W)
    elu(z)  = min(exp(z)-1+relu(z), relu(z))

Sharding: node dim N padded 50000 -> 50176 = 8 cores * 49 tiles * 128 rows.
Per 128-row tile the 2048 (n,k) pairs form 16 blocks of [128 nk-rows, 128 feat]
held as x_tile[:, b*128:(b+1)*128] (host pre-permutes x; all data bf16).
Padded rows are shipped UNMASKED (mask=0) so their softmax is uniform over
zero vectors -> output 0, no inf/NaN.

Per tile:
  DVE : 17x tensor_tensor_reduce (si + 16 score blocks; the per-block
        reduction is SEEDED with si+mask via the scalar-init AP, fusing the
        bias add), lrelu STT, reciprocal (reads PSUM), att STT, attseg TT
  PE  : si scatter + mask (2 accumulating matmuls), Z group-sum (SEG),
        RZ broadcast (E8), x0^T identity + 16 xbar matmuls (PSUM acc),
        final (xbar+x0)^T.T @ W          -- all bf16, single-pass + FWL
  ACT : si_s copy, Exp(p), ST copy, Exp/Relu of final
  GPS : Dt = SEG8*si, elu tail (e-1+r, min)
"""

import numpy as np

N, K, F = 50000, 16, 128
ALPHA = 0.2
NCORES = 8
TILE = 128
NTILES = 49
RPC = TILE * NTILES          # rows per core = 6272
BPT = K                      # nk-blocks per tile = 16
XCOLS = BPT * F + F + K      # x blocks + x0 + mask = 2192
MASKVAL = -700.0             # exp(0.2*(s-700)) underflows to 0 for any real s
EPS = 1e-12

_NC_CACHE = {}


def _register_scan_op():
    """Register the fused multiply+prefix-sum DVE op (1 elem/cycle, one
    instruction for a whole tile's score products). Idempotent."""
    from concourse import dve_ops
    from concourse.dve_ops import (
        CUSTOM_DVE_SPECS, OPS, _CUSTOM_DVE_ROW_BASE, _SUB_OPCODE_FOR_NAME,
        DveOp,
    )
    from concourse.dve_spec import AluOp, Spec, Src0, Src1, lower, scan
    from concourse.dve_uop import DveOpSpec

    name = "GAT_MUL_SCAN"
    if name in _SUB_OPCODE_FOR_NAME:
        return next(o for o in OPS if o.name == name)

    def _ref(in0, in1, c0, c1, c2):
        P = in0.shape[0]
        a0 = np.asarray(in0).astype(np.float32).reshape(P, -1)
        a1 = np.asarray(in1).astype(np.float32).reshape(P, -1)
        if a1.shape[1] != a0.shape[1]:
            a1 = np.tile(a1, (1, a0.shape[1] // a1.shape[1]))
        return np.cumsum(a0 * a1, axis=1).reshape(np.asarray(in0).shape)

    spec = Spec(body=scan(AluOp.ADD, Src0 * Src1), reference=_ref)
    opcode = _CUSTOM_DVE_ROW_BASE + len(OPS)
    shas = {}
    for ver in ("v3", "v4"):
        tmp = DveOpSpec(name=name, opcode=opcode, uops=lower(spec, ver=ver),
                        rd1_en=True)
        shas[ver] = tmp.sha(ver)
    op = DveOp(name, spec, subdim=False, uops_sha=shas)
    OPS.append(op)
    _SUB_OPCODE_FOR_NAME[name] = opcode
    CUSTOM_DVE_SPECS[name] = spec
    return op


def _consts_np():
    import ml_dtypes
    p = np.arange(128)
    j8 = np.arange(8)
    b16 = np.arange(16)
    ident = np.eye(128, dtype=np.float32)
    # Cm[n, q] = 1 iff n%8 == q//16   (si scatter: si_s[q,b] = si[8b + q//16])
    Cm = (p[:, None] % 8 == p[None, :] // 16).astype(np.float32)
    # SEGBIG[q, 8b+j] = 1 iff j == q//16  (pattern repeats over b)
    segbig = (p[:, None] // 16 == (p[None, :] % 8)).astype(np.float32)
    # E8[j, q] = 1 iff q//16 == j (rows 8..127 zero; used as lhsT [8,128])
    e8 = ((p[:, None] < 8) & (p[None, :] // 16 == p[:, None])).astype(np.float32)
    # SEG[q, j] = 1 iff q//16 == j   [128, 8]
    seg = (p[:, None] // 16 == j8[None, :]).astype(np.float32)
    # SEG8[n, b] = 1 iff n//8 == b   [128, 16]
    seg8 = (p[:, None] // 8 == b16[None, :]).astype(np.float32)
    return np.concatenate([ident, Cm, segbig, e8, seg, seg8], axis=1)  # [128, 536]


def _consts_full_np(W, a):
    import ml_dtypes
    bf16 = ml_dtypes.bfloat16
    W64 = np.asarray(W, np.float64)
    wa1 = (W64 @ np.asarray(a[:F, 0], np.float64)).astype(np.float32)
    wa2 = (W64 @ np.asarray(a[F:, 0], np.float64)).astype(np.float32)
    wa1_rep = np.broadcast_to(wa1[None, :], (128, F))
    wa2_rep = np.broadcast_to(wa2[None, :], (128, F))
    return np.ascontiguousarray(
        np.concatenate(
            [_consts_np(), np.asarray(W, np.float32), wa1_rep, wa2_rep,
             wa1[:, None]], axis=1)
    ).astype(bf16)  # [128, 921]


def _build_nc(ntiles=NTILES, finalize=True):
    import concourse.mybir as mybir
    import concourse.tile as tile
    from concourse import bacc

    fp = mybir.dt.float32
    bf = mybir.dt.bfloat16
    AF = mybir.ActivationFunctionType
    OP = mybir.AluOpType

    scan_op = _register_scan_op()

    nc = bacc.Bacc("TRN2")
    xd = nc.dram_tensor("xd", [ntiles, 128, XCOLS], bf, kind="ExternalInput")
    cst = nc.dram_tensor("cst", [128, 921], bf, kind="ExternalInput")
    yd = nc.dram_tensor("yd", [ntiles, 128, F], bf, kind="ExternalOutput")

    with tile.TileContext(nc) as tc:
        with (
            tc.tile_pool(name="const", bufs=1) as constp,
            tc.tile_pool(name="xin", bufs=7) as xin,
            tc.tile_pool(name="small", bufs=4) as small,
            tc.tile_pool(name="big", bufs=3) as big,
            tc.tile_pool(name="yout", bufs=3) as yout,
            # one PSUM pool; per-tag bufs: si 1 + Z 2 + RZrep 2 + mm 3 = 8 banks
            tc.tile_pool(name="ps", bufs=1, space="PSUM") as ps,
        ):
            consts = constp.tile([128, 921], bf)
            nc.sync.dma_start(out=consts, in_=cst[:, :])
            IDENT = consts[:, 0:128]
            Cm = consts[:, 128:256]
            SEGBIG = consts[:, 256:384]
            E8 = consts[:, 384:512]
            SEG = consts[:, 512:520]
            SEG8 = consts[:, 520:536]
            W_sb = consts[:, 536:664]
            wa1_rep = consts[:, 664:792]
            wa2_rep = consts[:, 792:920]
            wa1_col = consts[:, 920:921]

            # scan output buffers: col 0 is a permanent zero guard so the
            # per-block sums fall out of one strided subtract
            scan_bufs = []
            for i in range(2):
                sb_ = constp.tile([128, BPT * F + 1], fp, tag=f"scan{i}")
                nc.vector.memset(sb_[:, 0:1], 0.0)
                scan_bufs.append(sb_)

            # ------------- software-pipelined tile loop -------------
            #   load(t) | score(t-2) | mask(t-3) | recip(t-4) | out(t-5)
            st = {}

            def phase_load(t):
                xall = xin.tile([128, XCOLS], bf, tag="x")
                nc.sync.dma_start(out=xall, in_=xd[t])
                st[t] = {"xall": xall}

            def phase_score(t):
                d = st[t]
                xall = d["xall"]
                x0T_tile = xall[:, BPT * F:BPT * F + F]
                adjm = xall[:, BPT * F + F:XCOLS]
                # si[n] = x0[n]·wa1 on PE: lhsT = x0T [f, n], rhs = wa1 col
                si_ps = ps.tile([128, K + 1], fp, tag="si", bufs=2)
                nc.tensor.matmul(si_ps[:, K:K + 1], lhsT=x0T_tile, rhs=wa1_col,
                                 start=True, stop=True, skip_group_check=True)
                Dt = small.tile([128, K], bf, tag="D")
                nc.vector.tensor_scalar_mul(out=Dt, in0=SEG8,
                                            scalar1=si_ps[:, K:K + 1])
                nc.tensor.matmul(si_ps[:, 0:K], lhsT=Cm, rhs=Dt, start=True,
                                 stop=False, skip_group_check=True)
                nc.tensor.matmul(si_ps[:, 0:K], lhsT=IDENT, rhs=adjm,
                                 start=False, stop=True, skip_group_check=True)
                si_s = small.tile([128, K], fp, tag="si_s")
                nc.scalar.activation(out=si_s, in_=si_ps[:, 0:K], func=AF.Copy)

                sc = scan_bufs[t % 2]
                wa2_bc = wa2_rep.rearrange("p (o f) -> p o f", o=1).to_broadcast(
                    [128, BPT, F])
                nc.vector._custom_dve(
                    scan_op,
                    out=sc[:, 1:BPT * F + 1].rearrange("p (b f) -> p b f", f=F),
                    in0=xall[:, 0:BPT * F].rearrange("p (b f) -> p b f", f=F),
                    in1=wa2_bc,
                )
                ends = sc[:, 1:BPT * F + 1].rearrange(
                    "p (b f) -> p b f", f=F)[:, :, F - 1:F]
                starts = sc[:, 0:BPT * F].rearrange(
                    "p (b f) -> p b f", f=F)[:, :, 0:1]
                s = small.tile([128, K], fp, tag="s")
                nc.vector.scalar_tensor_tensor(
                    out=s.rearrange("p (b o) -> p b o", o=1),
                    in0=ends, scalar=1.0, in1=starts,
                    op0=OP.mult, op1=OP.subtract,
                )
                s2 = small.tile([128, K], fp, tag="s2")
                nc.gpsimd.tensor_add(out=s2, in0=s, in1=si_s)
                d["s2"] = s2

            def phase_mask(t):
                d = st[t]
                ls = small.tile([128, K], fp, tag="ls")
                nc.vector.scalar_tensor_tensor(
                    out=ls, in0=d["s2"], scalar=ALPHA, in1=d["s2"],
                    op0=OP.mult, op1=OP.max,
                )
                p_s = small.tile([128, K], bf, tag="p_s")
                nc.scalar.activation(out=p_s, in_=ls, func=AF.Exp)
                Z_ps = ps.tile([8, K], fp, tag="Z", bufs=2)
                nc.tensor.matmul(Z_ps, lhsT=SEG, rhs=p_s, start=True, stop=True)
                d["p_s"] = p_s
                d["Z_ps"] = Z_ps

            def phase_recip(t):
                d = st[t]
                RZ = small.tile([8, K], bf, tag="RZ")
                with nc.allow_low_precision(reason="RZ bf16 feeds bf16 matmul"):
                    nc.vector.reciprocal(RZ, d["Z_ps"])
                RZrep_ps = ps.tile([128, K], fp, tag="RZrep", bufs=2)
                nc.tensor.matmul(RZrep_ps, lhsT=E8[0:8, :], rhs=RZ,
                                 start=True, stop=True)
                d["RZrep"] = RZrep_ps

            def phase_out(t):
                d = st[t]
                xall = d["xall"]
                x0T_tile = xall[:, BPT * F:BPT * F + F]
                att = small.tile([128, K], bf, tag="att")
                nc.vector.scalar_tensor_tensor(
                    out=att, in0=d["p_s"], scalar=EPS, in1=d["RZrep"],
                    op0=OP.add, op1=OP.mult,
                )
                # previous tile's elu tail sits between att and attseg so the
                # DVE pipe-drain of att is hidden
                prev = st.get(t - 1)
                if prev is not None and "e2" in prev:
                    y_sb = yout.tile([128, 128], bf, tag="y")
                    nc.vector.scalar_tensor_tensor(
                        out=y_sb, in0=prev["e2"], scalar=-1.0, in1=prev["r"],
                        op0=OP.add, op1=OP.add,
                    )
                    nc.sync.dma_start(out=yd[t - 1], in_=y_sb)
                    del prev["e2"], prev["r"]
                attseg = big.tile([128, 128], bf, tag="attseg")
                att_bc = att.rearrange("p (b o) -> p b o", o=1).to_broadcast([128, K, 8])
                nc.vector.tensor_mul(
                    out=attseg.rearrange("p (b j) -> p b j", j=8),
                    in0=SEGBIG.rearrange("p (b j) -> p b j", j=8),
                    in1=att_bc,
                )
                xbarT_ps = ps.tile([128, 128], fp, tag="mm", bufs=2)
                nc.tensor.matmul(xbarT_ps, lhsT=IDENT, rhs=x0T_tile,
                                 start=True, stop=False, skip_group_check=True)
                for b in range(BPT):
                    nc.tensor.matmul(
                        xbarT_ps[:, 8 * b:8 * b + 8],
                        lhsT=xall[:, b * F:(b + 1) * F],
                        rhs=attseg[:, 8 * b:8 * b + 8],
                        start=False, stop=(b == BPT - 1), skip_group_check=True,
                    )
                ST_sb = big.tile([128, 128], bf, tag="ST")
                nc.scalar.activation(out=ST_sb, in_=xbarT_ps, func=AF.Copy)
                zfin_ps = ps.tile([128, 128], fp, tag="mm", bufs=2)
                nc.tensor.matmul(zfin_ps, lhsT=ST_sb, rhs=W_sb, start=True, stop=True)
                r_sb = big.tile([128, 128], bf, tag="r")
                nc.scalar.activation(out=r_sb, in_=zfin_ps, func=AF.Relu)
                rm_sb = big.tile([128, 128], fp, tag="rm")
                nc.scalar.activation(out=rm_sb, in_=zfin_ps, func=AF.Relu,
                                     scale=-1.0)
                e2_sb = big.tile([128, 128], bf, tag="e2")
                nc.scalar.activation(out=e2_sb, in_=rm_sb, func=AF.Exp,
                                     scale=-1.0)
                d["e2"] = e2_sb
                d["r"] = r_sb
                # y of this tile is emitted by phase_out(t+1) (or the drain
                # below for the final tile)

            for r in range(ntiles + 5):
                if r < ntiles:
                    phase_load(r)
                if 0 <= r - 2 < ntiles:
                    phase_score(r - 2)
                if 0 <= r - 3 < ntiles:
                    phase_mask(r - 3)
                if 0 <= r - 4 < ntiles:
                    phase_recip(r - 4)
                if 0 <= r - 5 < ntiles:
                    phase_out(r - 5)
            last = st[ntiles - 1]
            y_sb = yout.tile([128, 128], bf, tag="y")
            nc.vector.scalar_tensor_tensor(
                out=y_sb, in0=last["e2"], scalar=-1.0, in1=last["r"],
                op0=OP.add, op1=OP.add,
            )
            nc.sync.dma_start(out=yd[ntiles - 1], in_=y_sb)

    if finalize:
        nc.finalize()
    return nc


def _get_nc(ntiles=NTILES):
    if ntiles not in _NC_CACHE:
        _NC_CACHE[ntiles] = _build_nc(ntiles)
    return _NC_CACHE[ntiles]


def _shard_inputs(orignal_x, x, adj, W, a, ncores=NCORES, ntiles=NTILES):
    import ml_dtypes
    bf16 = ml_dtypes.bfloat16
    f32 = np.float32
    rpc = TILE * ntiles
    n_used = rpc * ncores
    x = np.asarray(x, f32)
    x0 = np.asarray(orignal_x, f32)
    adj = np.asarray(adj, np.int32)
    consts = _consts_full_np(np.asarray(W, f32), np.asarray(a, f32))
    n = x.shape[0]

    in_maps = []
    for c in range(ncores):
        lo = c * rpc
        hi = min((c + 1) * rpc, n)
        rows = hi - lo
        xc = x[lo:hi]
        x0c = x0[lo:hi]
        # mask: 0 where active, -700 where masked; padded rows UNMASKED (0)
        mc = np.where(adj[lo:hi] > 0, 0.0, MASKVAL).astype(f32)
        if rows < rpc:
            pad = rpc - rows
            xc = np.concatenate([xc, np.zeros((pad, K, F), f32)])
            x0c = np.concatenate([x0c, np.zeros((pad, F), f32)])
            mc = np.concatenate([mc, np.zeros((pad, K), f32)])
        xdev = np.empty((ntiles, 128, XCOLS), bf16)
        xdev[:, :, :BPT * F] = xc.reshape(ntiles, BPT, 128, F).transpose(
            0, 2, 1, 3).reshape(ntiles, 128, BPT * F).astype(bf16)
        xdev[:, :, BPT * F:BPT * F + F] = x0c.reshape(
            ntiles, 128, F).transpose(0, 2, 1).astype(bf16)
        xdev[:, :, BPT * F + F:] = mc.reshape(ntiles, BPT, 128).transpose(
            0, 2, 1).astype(bf16)
        in_maps.append({
            "xd": xdev,
            "cst": consts,
        })
    assert n <= n_used
    return in_maps


_LAST_RESULTS = None


def kernel(orignal_x, x, adj, W, a):
    import os
    os.environ.setdefault("JAX_PLATFORMS", "")
    from concourse.bass_utils import run_bass_kernel_spmd

    global _LAST_RESULTS
    nc = _get_nc()
    in_maps = _shard_inputs(orignal_x, x, adj, W, a)
    res = run_bass_kernel_spmd(nc, in_maps, list(range(NCORES)))
    _LAST_RESULTS = res
    y = np.concatenate([r["yd"].reshape(RPC, F) for r in res.results], axis=0)
    return np.ascontiguousarray(y[:N].astype(np.float32))
